# revision 29
# baseline (speedup 1.0000x reference)
"""GAT attention head (nn_AttHead) on 8 Trainium2 NeuronCores.

Reference computation:
    h = input @ W                  [N, F]
    e_ij = leakyrelu(f_src_i + f_dst_j, 0.2);  masked softmax over j (mask=adj)
    h' = elu(softmax(e) @ h)

Key restructuring used here (exact algebra, not an approximation):
    exp(lrelu(s)) = exp(0.2 s) * max(exp(0.8 s), 1)
    s_ij = f_src_i + f_dst_j is rank-1, so with
        u_i = exp(0.8 f_src_i), v_j = exp(0.8 f_dst_j), q_j = exp(0.2 f_dst_j)
    the row factor exp(0.2 f_src_i) cancels in the softmax and
        att_ij ∝ A_ij * q_j * max(u_i v_j, 1)
        h'_i = (Σ_j A_ij max(u_i v_j,1) [q_j h_j, q_j]) / (denominator column)
    This removes every transcendental from the O(N^2) inner loop: per tile the
    device only needs one tensor_scalar (mult+max), one tensor_tensor (mask
    multiply) and a matmul accumulation; u, v, q are O(N) host precomputes.

Sharding: row-parallel over the N=8192 output rows; core c owns rows
[c*1024, (c+1)*1024). Scores are built transposed ([j on partitions, i free])
so the PE can contract over j directly. The adjacency mask is shipped as a
bf16 {0,1} matrix transposed to [j, i] layout (host-side data marshaling).
"""

import os
import numpy as np
import ml_dtypes

N = 8192
IN_F = 128
OUT_F = 64
HT_F = OUT_F + 1  # h-tilde carries a denominator ones-column (scaled by q)
N_CORES = 8
SLAB = N // N_CORES  # 1024 output rows per core
P = 128
NT = N // P  # 64 j-chunks of 128
HALF = SLAB // 2  # PSUM free-dim limit for fp32 output is 512

_bf16 = ml_dtypes.bfloat16

_nc_cache = None
_DVE_MOD = int(os.environ.get("KCFG_DVE_MOD", "2"))


def _build_bass():
    import concourse.mybir as mybir
    import concourse.tile as tile
    from concourse import bacc

    bf = mybir.dt.bfloat16
    f32 = mybir.dt.float32
    Alu = mybir.AluOpType

    nc = bacc.Bacc("TRN2", target_bir_lowering=False, debug=False)
    global _DVE_MOD

    maskT = nc.dram_tensor("maskT", [N, SLAB], bf, kind="ExternalInput")
    u_bc = nc.dram_tensor("u_bc", [P, SLAB], bf, kind="ExternalInput")
    vT = nc.dram_tensor("vT", [P, NT], f32, kind="ExternalInput")
    vTn = nc.dram_tensor("vTn", [P, NT], f32, kind="ExternalInput")
    ht = nc.dram_tensor("ht", [P, NT * HT_F], bf, kind="ExternalInput")
    htv = nc.dram_tensor("htv", [P, NT * HT_F], bf, kind="ExternalInput")
    out = nc.dram_tensor("out", [OUT_F, SLAB], f32, kind="ExternalOutput")

    maskT_t = maskT.rearrange("(t p) i -> t p i", p=P)

    with tile.TileContext(nc) as tc:
        with (
            tc.tile_pool(name="const", bufs=1) as cpool,
            tc.tile_pool(name="mask", bufs=8) as mpool,
            tc.tile_pool(name="gt", bufs=6) as gpool,
            tc.tile_pool(name="ps", bufs=1, space="PSUM") as pspool,
            tc.tile_pool(name="epi", bufs=1) as epool,
        ):
            vT_sb = cpool.tile([P, NT], f32)
            nc.sync.dma_start(vT_sb[:], vT[:])
            u_sb = cpool.tile([P, SLAB], bf)
            nc.sync.dma_start(u_sb[:], u_bc[:])
            vTn_sb = cpool.tile([P, NT], f32)
            nc.scalar.dma_start(vTn_sb[:], vTn[:])
            ht_sb = cpool.tile([P, NT, HT_F], bf)
            nc.scalar.dma_start(ht_sb[:], ht.rearrange("p (t f) -> p t f", f=HT_F))
            htv_sb = cpool.tile([P, NT, HT_F], bf)
            nc.scalar.dma_start(htv_sb[:], htv.rearrange("p (t f) -> p t f", f=HT_F))

            # Warm the ACT exp table during the main loop (ScalarE is idle);
            # output is unused.
            warm = cpool.tile([P, 8], f32)
            nc.scalar.activation(
                warm[:], u_sb[:, 0:8], mybir.ActivationFunctionType.Exp
            )

            ps0 = pspool.tile([HT_F, HALF], f32)
            ps1 = pspool.tile([HT_F, HALF], f32)

            # Score tiles, [j on partitions, i free]:   p = max(u'_i, v_j) * A
            # Two equivalent forms, split across engines so DVE and ACT share
            # the elementwise load:
            #   DVE form:  g = max(u', v_j)      (tensor_scalar, VectorE)
            #              p = g * A             (tensor_tensor, VectorE)
            #              psum += ht_t.T @ p
            #   ACT form:  r = relu(u' - v_j)    (ScalarE; max(u',v) = v + r)
            #              p = r * A             (tensor_tensor, VectorE)
            #              psum += ht_t.T @ p + htv_t.T @ A
            #   (htv = v-scaled ht supplies the "+ v_j" term through the mask)
            # Groups of GRP chunks share one mask DMA and one batched TT; the
            # first 4 chunks run as singletons to prime the pipeline.
            groups = [(t, 1, "dve") for t in range(4)]
            NGRP = (NT - 4) // 4
            for k in range(NGRP):
                form = "dve" if k % _DVE_MOD == 1 else "act"
                groups.append((4 + 4 * k, 4, form))

            for t0g, grp, form in groups:
                m4 = mpool.tile([P, 4, SLAB], bf, tag="m4")
                nc.sync.dma_start(
                    m4[:, 0:grp, :],
                    maskT_t[t0g : t0g + grp].rearrange("t p i -> p t i"),
                )
                g4 = gpool.tile([P, 4, SLAB], bf, tag="g4")
                for b in range(grp):
                    t = t0g + b
                    if form == "dve":
                        nc.vector.tensor_scalar(
                            g4[:, b, :], u_sb[:], vT_sb[:, t : t + 1], None, Alu.max
                        )
                    else:
                        nc.scalar.activation(
                            out=g4[:, b, :],
                            in_=u_sb[:],
                            func=mybir.ActivationFunctionType.Relu,
                            bias=vTn_sb[:, t : t + 1],
                            scale=1.0,
                        )
                # mask multiply in place: g4 <- g4 * m4
                nc.vector.tensor_tensor(
                    g4[:, 0:grp, :], g4[:, 0:grp, :], m4[:, 0:grp, :], Alu.mult
                )
                for b in range(grp):
                    t = t0g + b
                    last = t == NT - 1
                    nc.tensor.matmul(
                        ps0[:],
                        ht_sb[:, t, :],
                        g4[:, b, 0:HALF],
                        start=(t == 0),
                        stop=(last and form == "dve"),
                    )
                    nc.tensor.matmul(
                        ps1[:],
                        ht_sb[:, t, :],
                        g4[:, b, HALF:SLAB],
                        start=(t == 0),
                        stop=(last and form == "dve"),
                    )
                    if form == "act":
                        nc.tensor.matmul(
                            ps0[:],
                            htv_sb[:, t, :],
                            m4[:, b, 0:HALF],
                            start=False,
                            stop=last,
                        )
                        nc.tensor.matmul(
                            ps1[:],
                            htv_sb[:, t, :],
                            m4[:, b, HALF:SLAB],
                            start=False,
                            stop=last,
                        )

            # ---- epilogue: divide by denominator row, then ELU ----
            num = epool.tile([HT_F, SLAB], f32)
            nc.vector.tensor_copy(out=num[:, 0:HALF], in_=ps0[:])
            nc.vector.tensor_copy(out=num[:, HALF:SLAB], in_=ps1[:])

            # Spread the 1024 denominators over all 128 partitions via an
            # SBUF->SBUF DMA so the iterative-divide reciprocal runs 128-wide,
            # then repack the result to a [1, 1024] row for the broadcast.
            den128 = epool.tile([P, SLAB // P], f32)
            nc.sync.dma_start(den128[:], num[OUT_F : OUT_F + 1, :])
            rcp128 = epool.tile([P, SLAB // P], f32)
            nc.vector.reciprocal(out=rcp128[:], in_=den128[:])
            rcp = epool.tile([1, SLAB], f32)
            nc.sync.dma_start(rcp[:], rcp128[:])

            # broadcast rcp across 64 partitions via a K=1 matmul with ones
            ones = epool.tile([1, OUT_F], f32)
            nc.vector.memset(ones[:], 1.0)
            pb0 = pspool.tile([OUT_F, HALF], f32)
            pb1 = pspool.tile([OUT_F, HALF], f32)
            nc.tensor.matmul(pb0[:], ones[:], rcp[:, 0:HALF])
            nc.tensor.matmul(pb1[:], ones[:], rcp[:, HALF:SLAB])

            div = epool.tile([OUT_F, SLAB], f32)
            nc.vector.tensor_tensor(
                div[:, 0:HALF], num[0:OUT_F, 0:HALF], pb0[:], Alu.mult
            )
            nc.vector.tensor_tensor(
                div[:, HALF:SLAB], num[0:OUT_F, HALF:SLAB], pb1[:], Alu.mult
            )

            # elu(x) = relu(x) + min(exp(x) - 1, 0)
            ex = epool.tile([OUT_F, SLAB], f32)
            nc.scalar.activation(ex[:], div[:], mybir.ActivationFunctionType.Exp)
            exm = epool.tile([OUT_F, SLAB], f32)
            nc.vector.tensor_scalar(
                exm[:], ex[:], 1.0, 0.0, Alu.subtract, Alu.min
            )
            rl = epool.tile([OUT_F, SLAB], f32)
            nc.vector.tensor_scalar(rl[:], div[:], 0.0, None, Alu.max)
            ov = epool.tile([OUT_F, SLAB], f32)
            nc.vector.tensor_tensor(ov[:], exm[:], rl[:], Alu.add)

            nc.sync.dma_start(out[:], ov[:])

    nc.finalize()
    return nc


def _get_nc():
    global _nc_cache
    if _nc_cache is None:
        _nc_cache = _build_bass()
    return _nc_cache


def prepare_inputs(input, adj, W, a):
    """Host-side O(N*F) precompute + input marshaling. Returns per-core input
    maps for the SPMD bass kernel."""
    f32 = np.float32
    input = np.asarray(input, dtype=f32)
    W = np.asarray(W, dtype=f32)
    a = np.asarray(a, dtype=f32)
    adj = np.asarray(adj)

    h = input @ W  # [N, 64]
    f_src = h @ a[:OUT_F]
    f_dst = h @ a[OUT_F:]

    u = np.exp(-0.8 * f_src).astype(_bf16)  # u' = exp(-0.8 f_src) per row i
    v = np.exp(0.8 * f_dst).astype(f32)  # [N] per neighbor j
    q = np.exp(0.2 * f_dst).astype(f32)

    htil = np.empty((N, HT_F), f32)
    htil[:, :OUT_F] = h * q[:, None]
    htil[:, OUT_F] = q

    def dev_layout(x):
        # partition p holds chunk t at columns [t*65, (t+1)*65)
        return np.ascontiguousarray(
            x.reshape(NT, P, HT_F).transpose(1, 0, 2).reshape(P, NT * HT_F)
        ).astype(_bf16)

    ht_dev = dev_layout(htil)
    htv_dev = dev_layout(htil * v[:, None])

    vT_dev = np.ascontiguousarray(v.reshape(NT, P).T)  # [128, 64] f32
    vTn_dev = np.ascontiguousarray(-vT_dev)

    # mask, transposed to [j, i], as bf16 {0.0, 1.0} via bit pattern
    m16 = (adj.T != 0).astype(np.uint16)
    m16 *= np.uint16(0x3F80)  # bf16 bits of 1.0
    maskT = m16.view(_bf16)  # [N(j), N(i)]

    in_maps = []
    for c in range(N_CORES):
        sl = slice(c * SLAB, (c + 1) * SLAB)
        in_maps.append(
            {
                "maskT": np.ascontiguousarray(maskT[:, sl]),
                "u_bc": np.ascontiguousarray(
                    np.broadcast_to(u[sl][None, :], (P, SLAB))
                ),
                "vT": vT_dev,
                "vTn": vTn_dev,
                "ht": ht_dev,
                "htv": htv_dev,
            }
        )
    return in_maps


def assemble_output(results):
    """results: list of 8 dicts with 'out' [64, 1024] f32 -> [N, 64] f32."""
    hp = np.empty((N, OUT_F), np.float32)
    for c in range(N_CORES):
        hp[c * SLAB : (c + 1) * SLAB] = results[c]["out"].T
    return hp


def kernel(input, adj, W, a):
    import time

    from concourse.bass_utils import run_bass_kernel_spmd

    nc = _get_nc()
    in_maps = prepare_inputs(input, adj, W, a)
    last_err = None
    for attempt in range(3):
        try:
            res = run_bass_kernel_spmd(nc, in_maps, core_ids=list(range(N_CORES)))
            return assemble_output(res.results)
        except Exception as e:  # transient device wedges have been observed
            last_err = e
            time.sleep(5)
    raise last_err


# revision 31
# speedup vs baseline: 1.0207x; 1.0207x over previous
"""GAT attention head (nn_AttHead_11330123727477) on 8 Trainium2 NeuronCores.

Reference computation:
    h = input @ W;  e_ij = leakyrelu(f_src_i + f_dst_j, 0.2)
    h' = elu(softmax_j(where(adj, e, -inf)) @ h)

Exact algebraic restructuring (no approximation beyond bf16 rounding):
    exp(lrelu(s)) = exp(0.2 s) * max(exp(0.8 s), 1), and s_ij = f_src_i+f_dst_j
    is rank-1. With u_i=exp(0.8 f_src_i), v_j=exp(0.8 f_dst_j), q_j=exp(0.2 f_dst_j),
    every per-row factor cancels in the softmax, and with u'_i = 1/u_i:
        att_ij ∝ A_ij * q_j * max(u'_i, v_j)        (after dividing row i by u_i)
        h'_i = (Σ_j A_ij max(u'_i,v_j) [q_j h_j, q_j]) / (denominator column)
    So the O(N^2) inner loop needs NO transcendentals - only max/multiply ops;
    u', v, q are O(N) host precomputes (the sharding hint's "replicate h" class).

Device mapping (per core, scores transposed to [j on partitions, i free]):
  - The 256 MB adjacency is shipped as a bf16 {0,1} mask, transposed to [j,i]
    (pure marshaling), streamed as 1.5 MB chunks.
  - Per 128xSLAB chunk, the score tile p = max(u'_i, v_j) * A is built one of
    two ways, statically split so VectorE and ScalarE share the load:
      DVE form:  g = max(u', v_j)   (tensor_scalar) ; p = g*A (tensor_tensor)
                 psum += ht_t.T @ p
      ACT form:  r = relu(u' - v_j) (ScalarE activation; max(u',v) = v_j + r)
                 p = r*A (tensor_tensor) ; psum += ht_t.T @ p + htv_t.T @ A
                 (htv = v-scaled ht carries the "+v_j" term via the raw mask)
  - PE contracts over j into a [65, i] PSUM accumulator whose 65th row is the
    softmax denominator (ones-column trick); epilogue divides, applies
    elu(x) = relu(x) + min(exp(x)-1, 0), and writes [64, SLAB] back.

Sharding: row-parallel over the 8192 output rows, 1024 rows per core,
no cross-core communication (per the sharding hint).
"""

import os
import numpy as np
import ml_dtypes

N = 8192
IN_F = 128
OUT_F = 64
HT_F = OUT_F + 1  # h-tilde carries a denominator ones-column (scaled by q)
N_CORES = 8
SLAB = N // N_CORES  # 1024 output rows per core
P = 128
NT = N // P  # 64 j-chunks of 128
HALF = SLAB // 2  # PSUM free-dim limit for fp32 output is 512

_bf16 = ml_dtypes.bfloat16

_nc_cache = None
_DVE_MOD = int(os.environ.get("KCFG_DVE_MOD", "2"))


def _build_bass():
    import concourse.mybir as mybir
    import concourse.tile as tile
    from concourse import bacc

    bf = mybir.dt.bfloat16
    f32 = mybir.dt.float32
    Alu = mybir.AluOpType

    nc = bacc.Bacc("TRN2", target_bir_lowering=False, debug=False)
    global _DVE_MOD

    maskT = nc.dram_tensor("maskT", [N, SLAB], bf, kind="ExternalInput")
    u_bc = nc.dram_tensor("u_bc", [P, SLAB], bf, kind="ExternalInput")
    vT = nc.dram_tensor("vT", [P, NT], f32, kind="ExternalInput")
    vTn = nc.dram_tensor("vTn", [P, NT], f32, kind="ExternalInput")
    ht = nc.dram_tensor("ht", [P, NT * HT_F], bf, kind="ExternalInput")
    htv = nc.dram_tensor("htv", [P, NT * HT_F], bf, kind="ExternalInput")
    out = nc.dram_tensor("out", [OUT_F, SLAB], f32, kind="ExternalOutput")

    maskT_t = maskT.rearrange("(t p) i -> t p i", p=P)

    with tile.TileContext(nc) as tc:
        with (
            tc.tile_pool(name="const", bufs=1) as cpool,
            tc.tile_pool(name="mask", bufs=5) as mpool,
            tc.tile_pool(name="gt", bufs=4) as gpool,
            tc.tile_pool(name="ps", bufs=1, space="PSUM") as pspool,
            tc.tile_pool(name="epi", bufs=1) as epool,
        ):
            vT_sb = cpool.tile([P, NT], f32)
            nc.sync.dma_start(vT_sb[:], vT[:])
            u_sb = cpool.tile([P, SLAB], bf)
            nc.sync.dma_start(u_sb[:], u_bc[:])
            vTn_sb = cpool.tile([P, NT], f32)
            nc.scalar.dma_start(vTn_sb[:], vTn[:])
            ht_sb = cpool.tile([P, NT, HT_F], bf)
            nc.scalar.dma_start(ht_sb[:], ht.rearrange("p (t f) -> p t f", f=HT_F))
            htv_sb = cpool.tile([P, NT, HT_F], bf)
            nc.scalar.dma_start(htv_sb[:], htv.rearrange("p (t f) -> p t f", f=HT_F))

            # Warm the ACT exp table during the main loop (ScalarE is idle);
            # output is unused.
            warm = cpool.tile([P, 8], f32)
            nc.scalar.activation(
                warm[:], u_sb[:, 0:8], mybir.ActivationFunctionType.Exp
            )

            ps0 = pspool.tile([HT_F, HALF], f32)
            ps1 = pspool.tile([HT_F, HALF], f32)

            # Score tiles, [j on partitions, i free]:   p = max(u'_i, v_j) * A
            # Two equivalent forms, split across engines so DVE and ACT share
            # the elementwise load:
            #   DVE form:  g = max(u', v_j)      (tensor_scalar, VectorE)
            #              p = g * A             (tensor_tensor, VectorE)
            #              psum += ht_t.T @ p
            #   ACT form:  r = relu(u' - v_j)    (ScalarE; max(u',v) = v + r)
            #              p = r * A             (tensor_tensor, VectorE)
            #              psum += ht_t.T @ p + htv_t.T @ A
            #   (htv = v-scaled ht supplies the "+ v_j" term through the mask)
            # Groups of GRP chunks share one mask DMA and one batched TT; the
            # first 4 chunks run as singletons to prime the pipeline.
            groups = [(t, 1, "dve") for t in range(4)]
            NGRP = (NT - 4) // 6
            for k in range(NGRP):
                form = "dve" if k % _DVE_MOD == 1 else "act"
                groups.append((4 + 6 * k, 6, form))

            for t0g, grp, form in groups:
                m4 = mpool.tile([P, 6, SLAB], bf, tag="m4")
                nc.sync.dma_start(
                    m4[:, 0:grp, :],
                    maskT_t[t0g : t0g + grp].rearrange("t p i -> p t i"),
                )
                g4 = gpool.tile([P, 6, SLAB], bf, tag="g4")
                for b in range(grp):
                    t = t0g + b
                    if form == "dve":
                        nc.vector.tensor_scalar(
                            g4[:, b, :], u_sb[:], vT_sb[:, t : t + 1], None, Alu.max
                        )
                    else:
                        nc.scalar.activation(
                            out=g4[:, b, :],
                            in_=u_sb[:],
                            func=mybir.ActivationFunctionType.Relu,
                            bias=vTn_sb[:, t : t + 1],
                            scale=1.0,
                        )
                # mask multiply in place: g4 <- g4 * m4
                nc.vector.tensor_tensor(
                    g4[:, 0:grp, :], g4[:, 0:grp, :], m4[:, 0:grp, :], Alu.mult
                )
                for b in range(grp):
                    t = t0g + b
                    last = t == NT - 1
                    nc.tensor.matmul(
                        ps0[:],
                        ht_sb[:, t, :],
                        g4[:, b, 0:HALF],
                        start=(t == 0),
                        stop=(last and form == "dve"),
                    )
                    nc.tensor.matmul(
                        ps1[:],
                        ht_sb[:, t, :],
                        g4[:, b, HALF:SLAB],
                        start=(t == 0),
                        stop=(last and form == "dve"),
                    )
                    if form == "act":
                        nc.tensor.matmul(
                            ps0[:],
                            htv_sb[:, t, :],
                            m4[:, b, 0:HALF],
                            start=False,
                            stop=last,
                        )
                        nc.tensor.matmul(
                            ps1[:],
                            htv_sb[:, t, :],
                            m4[:, b, HALF:SLAB],
                            start=False,
                            stop=last,
                        )

            # ---- epilogue: divide by denominator row, then ELU ----
            num = epool.tile([HT_F, SLAB], f32)
            nc.vector.tensor_copy(out=num[:, 0:HALF], in_=ps0[:])
            nc.vector.tensor_copy(out=num[:, HALF:SLAB], in_=ps1[:])

            # Spread the 1024 denominators over all 128 partitions via an
            # SBUF->SBUF DMA so the iterative-divide reciprocal runs 128-wide,
            # then repack the result to a [1, 1024] row for the broadcast.
            den128 = epool.tile([P, SLAB // P], f32)
            nc.sync.dma_start(den128[:], num[OUT_F : OUT_F + 1, :])
            rcp128 = epool.tile([P, SLAB // P], f32)
            nc.vector.reciprocal(out=rcp128[:], in_=den128[:])
            rcp = epool.tile([1, SLAB], f32)
            nc.sync.dma_start(rcp[:], rcp128[:])

            # broadcast rcp across 64 partitions via a K=1 matmul with ones
            ones = epool.tile([1, OUT_F], f32)
            nc.vector.memset(ones[:], 1.0)
            pb0 = pspool.tile([OUT_F, HALF], f32)
            pb1 = pspool.tile([OUT_F, HALF], f32)
            nc.tensor.matmul(pb0[:], ones[:], rcp[:, 0:HALF])
            nc.tensor.matmul(pb1[:], ones[:], rcp[:, HALF:SLAB])

            div = epool.tile([OUT_F, SLAB], f32)
            nc.vector.tensor_tensor(
                div[:, 0:HALF], num[0:OUT_F, 0:HALF], pb0[:], Alu.mult
            )
            nc.vector.tensor_tensor(
                div[:, HALF:SLAB], num[0:OUT_F, HALF:SLAB], pb1[:], Alu.mult
            )

            # elu(x) = relu(x) + min(exp(x) - 1, 0)
            ex = epool.tile([OUT_F, SLAB], f32)
            nc.scalar.activation(ex[:], div[:], mybir.ActivationFunctionType.Exp)
            exm = epool.tile([OUT_F, SLAB], f32)
            nc.vector.tensor_scalar(
                exm[:], ex[:], 1.0, 0.0, Alu.subtract, Alu.min
            )
            rl = epool.tile([OUT_F, SLAB], f32)
            nc.vector.tensor_scalar(rl[:], div[:], 0.0, None, Alu.max)
            ov = epool.tile([OUT_F, SLAB], f32)
            nc.vector.tensor_tensor(ov[:], exm[:], rl[:], Alu.add)

            nc.sync.dma_start(out[:], ov[:])

    nc.finalize()
    return nc


def _get_nc():
    global _nc_cache
    if _nc_cache is None:
        _nc_cache = _build_bass()
    return _nc_cache


def prepare_inputs(input, adj, W, a):
    """Host-side O(N*F) precompute + input marshaling. Returns per-core input
    maps for the SPMD bass kernel."""
    f32 = np.float32
    input = np.asarray(input, dtype=f32)
    W = np.asarray(W, dtype=f32)
    a = np.asarray(a, dtype=f32)
    adj = np.asarray(adj)

    h = input @ W  # [N, 64]
    f_src = h @ a[:OUT_F]
    f_dst = h @ a[OUT_F:]

    u = np.exp(-0.8 * f_src).astype(_bf16)  # u' = exp(-0.8 f_src) per row i
    v = np.exp(0.8 * f_dst).astype(f32)  # [N] per neighbor j
    q = np.exp(0.2 * f_dst).astype(f32)

    htil = np.empty((N, HT_F), f32)
    htil[:, :OUT_F] = h * q[:, None]
    htil[:, OUT_F] = q

    def dev_layout(x):
        # partition p holds chunk t at columns [t*65, (t+1)*65)
        return np.ascontiguousarray(
            x.reshape(NT, P, HT_F).transpose(1, 0, 2).reshape(P, NT * HT_F)
        ).astype(_bf16)

    ht_dev = dev_layout(htil)
    htv_dev = dev_layout(htil * v[:, None])

    vT_dev = np.ascontiguousarray(v.reshape(NT, P).T)  # [128, 64] f32
    vTn_dev = np.ascontiguousarray(-vT_dev)

    # mask, transposed to [j, i], as bf16 {0.0, 1.0} via bit pattern
    m16 = (adj.T != 0).astype(np.uint16)
    m16 *= np.uint16(0x3F80)  # bf16 bits of 1.0
    maskT = m16.view(_bf16)  # [N(j), N(i)]

    in_maps = []
    for c in range(N_CORES):
        sl = slice(c * SLAB, (c + 1) * SLAB)
        in_maps.append(
            {
                "maskT": np.ascontiguousarray(maskT[:, sl]),
                "u_bc": np.ascontiguousarray(
                    np.broadcast_to(u[sl][None, :], (P, SLAB))
                ),
                "vT": vT_dev,
                "vTn": vTn_dev,
                "ht": ht_dev,
                "htv": htv_dev,
            }
        )
    return in_maps


def assemble_output(results):
    """results: list of 8 dicts with 'out' [64, 1024] f32 -> [N, 64] f32."""
    hp = np.empty((N, OUT_F), np.float32)
    for c in range(N_CORES):
        hp[c * SLAB : (c + 1) * SLAB] = results[c]["out"].T
    return hp


def kernel(input, adj, W, a):
    import time

    from concourse.bass_utils import run_bass_kernel_spmd

    nc = _get_nc()
    in_maps = prepare_inputs(input, adj, W, a)
    last_err = None
    for attempt in range(3):
        try:
            res = run_bass_kernel_spmd(nc, in_maps, core_ids=list(range(N_CORES)))
            return assemble_output(res.results)
        except Exception as e:  # transient device wedges have been observed
            last_err = e
            time.sleep(5)
    raise last_err


# revision 32
# speedup vs baseline: 1.1814x; 1.1574x over previous
"""GAT attention head (nn_AttHead_11330123727477) on 8 Trainium2 NeuronCores.

Reference computation:
    h = input @ W;  e_ij = leakyrelu(f_src_i + f_dst_j, 0.2)
    h' = elu(softmax_j(where(adj, e, -inf)) @ h)

Exact algebraic restructuring (no approximation beyond bf16 rounding):
    exp(lrelu(s)) = exp(0.2 s) * max(exp(0.8 s), 1), and s_ij = f_src_i+f_dst_j
    is rank-1. With u_i=exp(0.8 f_src_i), v_j=exp(0.8 f_dst_j), q_j=exp(0.2 f_dst_j),
    every per-row factor cancels in the softmax, and with u'_i = 1/u_i:
        att_ij ∝ A_ij * q_j * max(u'_i, v_j)        (after dividing row i by u_i)
        h'_i = (Σ_j A_ij max(u'_i,v_j) [q_j h_j, q_j]) / (denominator column)
    So the O(N^2) inner loop needs NO transcendentals - only max/multiply ops;
    u', v, q are O(N) host precomputes (the sharding hint's "replicate h" class).

Device mapping (per core, scores transposed to [j on partitions, i free]):
  - The 256 MB adjacency is shipped as a bf16 {0,1} mask, transposed to [j,i]
    (pure marshaling), streamed as 1.5 MB chunks.
  - Per 128xSLAB chunk, the score tile p = max(u'_i, v_j) * A is built one of
    two ways, statically split so VectorE and ScalarE share the load:
      DVE form:  g = max(u', v_j)   (tensor_scalar) ; p = g*A (tensor_tensor)
                 psum += ht_t.T @ p
      ACT form:  r = relu(u' - v_j) (ScalarE activation; max(u',v) = v_j + r)
                 p = r*A (tensor_tensor) ; psum += ht_t.T @ p + htv_t.T @ A
                 (htv = v-scaled ht carries the "+v_j" term via the raw mask)
  - PE contracts over j into a [65, i] PSUM accumulator whose 65th row is the
    softmax denominator (ones-column trick); epilogue divides, applies
    elu(x) = relu(x) + min(exp(x)-1, 0), and writes [64, SLAB] back.

Sharding: row-parallel over the 8192 output rows, 1024 rows per core,
no cross-core communication (per the sharding hint).
"""

import os
import numpy as np
import ml_dtypes

N = 8192
IN_F = 128
OUT_F = 64
HT_F = OUT_F + 1  # h-tilde carries a denominator ones-column (scaled by q)
N_CORES = 8
SLAB = N // N_CORES  # 1024 output rows per core
P = 128
NT = N // P  # 64 j-chunks of 128
HALF = SLAB // 2  # PSUM free-dim limit for fp32 output is 512

_bf16 = ml_dtypes.bfloat16

_nc_cache = None
_DVE_MOD = int(os.environ.get("KCFG_DVE_MOD", "2"))


def _build_bass():
    import concourse.mybir as mybir
    import concourse.tile as tile
    from concourse import bacc

    bf = mybir.dt.bfloat16
    f32 = mybir.dt.float32
    Alu = mybir.AluOpType

    nc = bacc.Bacc("TRN2", target_bir_lowering=False, debug=False)
    global _DVE_MOD

    maskT = nc.dram_tensor("maskT", [N, SLAB], bf, kind="ExternalInput")
    u_bc = nc.dram_tensor("u_bc", [P, SLAB], bf, kind="ExternalInput")
    vT = nc.dram_tensor("vT", [P, NT], f32, kind="ExternalInput")
    vTn = nc.dram_tensor("vTn", [P, NT], f32, kind="ExternalInput")
    ht = nc.dram_tensor("ht", [P, NT * HT_F], bf, kind="ExternalInput")
    htv = nc.dram_tensor("htv", [P, NT * HT_F], bf, kind="ExternalInput")
    out = nc.dram_tensor("out", [OUT_F, SLAB], f32, kind="ExternalOutput")

    maskT_t = maskT.rearrange("(t p) i -> t p i", p=P)

    with tile.TileContext(nc) as tc:
        with (
            tc.tile_pool(name="const", bufs=1) as cpool,
            tc.tile_pool(name="mask", bufs=5) as mpool,
            tc.tile_pool(name="gt", bufs=4) as gpool,
            tc.tile_pool(name="ps", bufs=1, space="PSUM") as pspool,
            tc.tile_pool(name="epi", bufs=1) as epool,
        ):
            vT_sb = cpool.tile([P, NT], f32)
            nc.sync.dma_start(vT_sb[:], vT[:])
            u_sb = cpool.tile([P, SLAB], bf)
            nc.sync.dma_start(u_sb[:], u_bc[:])
            vTn_sb = cpool.tile([P, NT], f32)
            nc.scalar.dma_start(vTn_sb[:], vTn[:])
            ht_sb = cpool.tile([P, NT, HT_F], bf)
            nc.scalar.dma_start(ht_sb[:], ht.rearrange("p (t f) -> p t f", f=HT_F))
            htv_sb = cpool.tile([P, NT, HT_F], bf)
            nc.scalar.dma_start(htv_sb[:], htv.rearrange("p (t f) -> p t f", f=HT_F))

            # Warm the ACT exp table during the main loop (ScalarE is idle);
            # output is unused.
            warm = cpool.tile([P, 8], f32)
            nc.scalar.activation(
                warm[:], u_sb[:, 0:8], mybir.ActivationFunctionType.Exp
            )

            ps0 = pspool.tile([HT_F, HALF], f32)
            ps1 = pspool.tile([HT_F, HALF], f32)

            # Score tiles, [j on partitions, i free]:   p = max(u'_i, v_j) * A
            # Two equivalent forms, split across engines so DVE and ACT share
            # the elementwise load:
            #   DVE form:  g = max(u', v_j)      (tensor_scalar, VectorE)
            #              p = g * A             (tensor_tensor, VectorE)
            #              psum += ht_t.T @ p
            #   ACT form:  r = relu(u' - v_j)    (ScalarE; max(u',v) = v + r)
            #              p = r * A             (tensor_tensor, VectorE)
            #              psum += ht_t.T @ p + htv_t.T @ A
            #   (htv = v-scaled ht supplies the "+ v_j" term through the mask)
            # Groups of GRP chunks share one mask DMA and one batched TT; the
            # first 4 chunks run as singletons to prime the pipeline.
            # per-chunk engine assignment: within each 6-chunk group the
            # first 4 chunks build scores on ScalarE, the last 2 on VectorE,
            # so both engines stay busy concurrently (VectorE also runs every
            # group's batched mask-multiply).
            N_ACT = 4
            groups = [(t, 1) for t in range(4)]
            NGRP = (NT - 4) // 6
            for k in range(NGRP):
                groups.append((4 + 6 * k, 6))

            for t0g, grp in groups:
                m4 = mpool.tile([P, 6, SLAB], bf, tag="m4")
                nc.sync.dma_start(
                    m4[:, 0:grp, :],
                    maskT_t[t0g : t0g + grp].rearrange("t p i -> p t i"),
                )
                g4 = gpool.tile([P, 6, SLAB], bf, tag="g4")
                forms = ["act"] * N_ACT + ["dve"] * (grp - N_ACT) if grp > 1 else ["dve"]
                for b in range(grp):
                    t = t0g + b
                    if forms[b] == "dve":
                        nc.vector.tensor_scalar(
                            g4[:, b, :], u_sb[:], vT_sb[:, t : t + 1], None, Alu.max
                        )
                    else:
                        nc.scalar.activation(
                            out=g4[:, b, :],
                            in_=u_sb[:],
                            func=mybir.ActivationFunctionType.Relu,
                            bias=vTn_sb[:, t : t + 1],
                            scale=1.0,
                        )
                # mask multiply in place: g4 <- g4 * m4
                nc.vector.tensor_tensor(
                    g4[:, 0:grp, :], g4[:, 0:grp, :], m4[:, 0:grp, :], Alu.mult
                )
                for b in range(grp):
                    t = t0g + b
                    last = t == NT - 1
                    nc.tensor.matmul(
                        ps0[:],
                        ht_sb[:, t, :],
                        g4[:, b, 0:HALF],
                        start=(t == 0),
                        stop=(last and forms[b] == "dve"),
                    )
                    nc.tensor.matmul(
                        ps1[:],
                        ht_sb[:, t, :],
                        g4[:, b, HALF:SLAB],
                        start=(t == 0),
                        stop=(last and forms[b] == "dve"),
                    )
                    if forms[b] == "act":
                        nc.tensor.matmul(
                            ps0[:],
                            htv_sb[:, t, :],
                            m4[:, b, 0:HALF],
                            start=False,
                            stop=last,
                        )
                        nc.tensor.matmul(
                            ps1[:],
                            htv_sb[:, t, :],
                            m4[:, b, HALF:SLAB],
                            start=False,
                            stop=last,
                        )

            # ---- epilogue: divide by denominator row, then ELU ----
            num = epool.tile([HT_F, SLAB], f32)
            nc.vector.tensor_copy(out=num[:, 0:HALF], in_=ps0[:])
            nc.vector.tensor_copy(out=num[:, HALF:SLAB], in_=ps1[:])

            # Spread the 1024 denominators over all 128 partitions via an
            # SBUF->SBUF DMA so the iterative-divide reciprocal runs 128-wide,
            # then repack the result to a [1, 1024] row for the broadcast.
            den128 = epool.tile([P, SLAB // P], f32)
            nc.sync.dma_start(den128[:], num[OUT_F : OUT_F + 1, :])
            rcp128 = epool.tile([P, SLAB // P], f32)
            nc.vector.reciprocal(out=rcp128[:], in_=den128[:])
            rcp = epool.tile([1, SLAB], f32)
            nc.sync.dma_start(rcp[:], rcp128[:])

            # broadcast rcp across 64 partitions via a K=1 matmul with ones
            ones = epool.tile([1, OUT_F], f32)
            nc.vector.memset(ones[:], 1.0)
            pb0 = pspool.tile([OUT_F, HALF], f32)
            pb1 = pspool.tile([OUT_F, HALF], f32)
            nc.tensor.matmul(pb0[:], ones[:], rcp[:, 0:HALF])
            nc.tensor.matmul(pb1[:], ones[:], rcp[:, HALF:SLAB])

            div = epool.tile([OUT_F, SLAB], f32)
            nc.vector.tensor_tensor(
                div[:, 0:HALF], num[0:OUT_F, 0:HALF], pb0[:], Alu.mult
            )
            nc.vector.tensor_tensor(
                div[:, HALF:SLAB], num[0:OUT_F, HALF:SLAB], pb1[:], Alu.mult
            )

            # elu(x) = relu(x) + min(exp(x) - 1, 0)
            ex = epool.tile([OUT_F, SLAB], f32)
            nc.scalar.activation(ex[:], div[:], mybir.ActivationFunctionType.Exp)
            exm = epool.tile([OUT_F, SLAB], f32)
            nc.vector.tensor_scalar(
                exm[:], ex[:], 1.0, 0.0, Alu.subtract, Alu.min
            )
            rl = epool.tile([OUT_F, SLAB], f32)
            nc.vector.tensor_scalar(rl[:], div[:], 0.0, None, Alu.max)
            ov = epool.tile([OUT_F, SLAB], f32)
            nc.vector.tensor_tensor(ov[:], exm[:], rl[:], Alu.add)

            nc.sync.dma_start(out[:], ov[:])

    nc.finalize()
    return nc


def _get_nc():
    global _nc_cache
    if _nc_cache is None:
        _nc_cache = _build_bass()
    return _nc_cache


def prepare_inputs(input, adj, W, a):
    """Host-side O(N*F) precompute + input marshaling. Returns per-core input
    maps for the SPMD bass kernel."""
    f32 = np.float32
    input = np.asarray(input, dtype=f32)
    W = np.asarray(W, dtype=f32)
    a = np.asarray(a, dtype=f32)
    adj = np.asarray(adj)

    h = input @ W  # [N, 64]
    f_src = h @ a[:OUT_F]
    f_dst = h @ a[OUT_F:]

    u = np.exp(-0.8 * f_src).astype(_bf16)  # u' = exp(-0.8 f_src) per row i
    v = np.exp(0.8 * f_dst).astype(f32)  # [N] per neighbor j
    q = np.exp(0.2 * f_dst).astype(f32)

    htil = np.empty((N, HT_F), f32)
    htil[:, :OUT_F] = h * q[:, None]
    htil[:, OUT_F] = q

    def dev_layout(x):
        # partition p holds chunk t at columns [t*65, (t+1)*65)
        return np.ascontiguousarray(
            x.reshape(NT, P, HT_F).transpose(1, 0, 2).reshape(P, NT * HT_F)
        ).astype(_bf16)

    ht_dev = dev_layout(htil)
    htv_dev = dev_layout(htil * v[:, None])

    vT_dev = np.ascontiguousarray(v.reshape(NT, P).T)  # [128, 64] f32
    vTn_dev = np.ascontiguousarray(-vT_dev)

    # mask, transposed to [j, i], as bf16 {0.0, 1.0} via bit pattern
    m16 = (adj.T != 0).astype(np.uint16)
    m16 *= np.uint16(0x3F80)  # bf16 bits of 1.0
    maskT = m16.view(_bf16)  # [N(j), N(i)]

    in_maps = []
    for c in range(N_CORES):
        sl = slice(c * SLAB, (c + 1) * SLAB)
        in_maps.append(
            {
                "maskT": np.ascontiguousarray(maskT[:, sl]),
                "u_bc": np.ascontiguousarray(
                    np.broadcast_to(u[sl][None, :], (P, SLAB))
                ),
                "vT": vT_dev,
                "vTn": vTn_dev,
                "ht": ht_dev,
                "htv": htv_dev,
            }
        )
    return in_maps


def assemble_output(results):
    """results: list of 8 dicts with 'out' [64, 1024] f32 -> [N, 64] f32."""
    hp = np.empty((N, OUT_F), np.float32)
    for c in range(N_CORES):
        hp[c * SLAB : (c + 1) * SLAB] = results[c]["out"].T
    return hp


def kernel(input, adj, W, a):
    import time

    from concourse.bass_utils import run_bass_kernel_spmd

    nc = _get_nc()
    in_maps = prepare_inputs(input, adj, W, a)
    last_err = None
    for attempt in range(3):
        try:
            res = run_bass_kernel_spmd(nc, in_maps, core_ids=list(range(N_CORES)))
            return assemble_output(res.results)
        except Exception as e:  # transient device wedges have been observed
            last_err = e
            time.sleep(5)
    raise last_err
